# revision 14
# baseline (speedup 1.0000x reference)
"""Multi-head attention (B=2, N=4096, D=512, H=8) on 8 trn2 NeuronCores.

Sharding: core c handles batch b = c//4 and head-pair p = c%4 (heads 2p,
2p+1).  Each core projects its batch's Q/K/V against its pair's weight
columns, computes transposed attention scores sT = K_h @ Q_h^T, applies
exp((1/sqrt(dk))*sT) split between the ACT engine (table exp, ~75% of
columns) and the DVE (~25%: a Schraudolph int16 bit-trick pass plus a
custom 7-stage correction op that multiplies by a quadratic in the
mantissa, sigma ~0.2%), multiplies by an augmented V (extra ones column,
M=65) so the softmax denominators fall out of the same matmul, and
applies its rows of Wo with both heads row-tiled concurrently in the PE
array.

Schedule notes (all for keeping the PE busy and HAM un-throttled):
  - K/V projections first (interleaved), then per-qc Q projections and
    the out-projection are woven into the attention stream.
  - attn@V matmuls are issued one k-block behind the scores matmuls so
    the PE never head-of-line blocks on exp results.
  - all HBM transfers are whole contiguous [128,512] tiles (host
    pre-tiles), minimizing DMA descriptor overhead.
Normalization by the softmax denominator commutes with the output
projection, so it is applied on the host during the cross-core
reduction.

Device layouts (host pre-arranges):
  xt{q,k,v}: [8, 4, 128, 512]  tile (nk, dc): X^T[dc*128:+128, nk*512:+512]
  w{q,k,v}:  [128, 4, 128]     w[p, dc, c] = W[dc*128+p, off+c]
  b{q,k,v}:  [128, 1]          pair slice of bias
  wo:        [128, 4, 128]     wo[p, mt, c] = Wo[off+p, mt*128+c]
Outputs per core:
  y0, y1: [4, 8, 128, 512]  tile (mt, qc): y^T[mt*128:+128, qc*512:+512]
  den:    [2, N]            softmax denominators per head
Final host step: out[b] = (sum_{p,h} y_h / den_h).T + bo
"""

import numpy as np

_B, _N, _D, _H, _DK = 2, 4096, 512, 8, 64
_NCORES = 8

_LN2 = float(np.log(2.0))
_A_DVE = 0.125 * 1024.0 / _LN2
_B_DVE = 15360.0
_CORR_A = -1.4763417585548537
_CORR_Q2 = 0.22711289921196798
_CORR_C = 0.9424678640725361

_nc_cache = {}
_exp_corr_op = None


def _get_exp_corr_op():
    """Register (once) the custom DVE op: out = ((u+C0)^2*C1 + C2) * Src0
    with u = bitwise_or(bitwise_and(Src0, Src1), 1.0f) — Src1 carries the
    fp32 mantissa mask 0x007FFFFF as a full-width tensor ([P,1] broadcast
    Src1 hangs the DVE on this runtime)."""
    global _exp_corr_op
    if _exp_corr_op is not None:
        return _exp_corr_op
    from concourse import dve_ops
    from concourse.dve_spec import (
        AluOp,
        Bin,
        C0,
        C1,
        C2,
        One,
        Spec,
        Src0,
        Src1,
        lower,
        sq,
    )
    from concourse.dve_uop import DveOpSpec

    name = "EXP16_CORR_ANT"
    for op in dve_ops.OPS:
        if op.name == name:
            _exp_corr_op = op
            return op

    u = Bin(AluOp.BITWISE_OR, Bin(AluOp.BITWISE_AND, Src0, Src1), One)
    body = (sq(u + C0) * C1 + C2) * Src0

    def _ref(in0, in1, s0, s1, imm2):
        b = np.asarray(in0, np.float32).view(np.uint32)
        m = np.asarray(in1, np.float32).view(np.uint32)
        uu = ((b & m) | np.uint32(0x3F800000)).view(np.float32)
        return ((uu + s0) ** 2 * s1 + imm2) * in0

    spec = Spec(body=body, reference=_ref)
    sha = {
        ver: DveOpSpec(name=name, uops=lower(spec, ver=ver)).sha(ver)
        for ver in ("v3", "v4")
    }
    op = dve_ops.DveOp(name, spec, subdim=False, uops_sha=sha)
    idx = len(dve_ops.OPS)
    dve_ops.OPS.append(op)
    dve_ops.CUSTOM_DVE_SPECS[name] = spec
    dve_ops._SUB_OPCODE_FOR_NAME[name] = dve_ops._CUSTOM_DVE_ROW_BASE + idx
    _exp_corr_op = op
    return op


def _build(n=_N, zero_bias=False, dve_split=True):
    import concourse.mybir as mybir
    import concourse.tile as tile
    from concourse import bacc
    from concourse.masks import make_identity

    f32 = mybir.dt.float32
    i16 = mybir.dt.int16
    i32 = mybir.dt.int32
    bf16 = mybir.dt.float16
    Exp = mybir.ActivationFunctionType.Exp
    NKC = n // 128  # k chunks of 128 (columns of sT)
    NQC = n // 512  # q chunks of 512
    BL = 2  # k-chunks per exp block
    blocks = []
    i = 0
    while i < NKC:
        blen = min(BL, NKC - i)
        blocks.append((i, blen))
        i += blen

    corr_op = _get_exp_corr_op() if dve_split else None

    def act_cols(blen):
        if not dve_split:
            return blen * 512
        return (int(blen * 512 * 0.75) // 32) * 32

    nc = bacc.Bacc(
        "TRN2", target_bir_lowering=False, debug=False, num_devices=_NCORES
    )

    xt = {
        t: nc.dram_tensor(f"xt{t}", [NQC, 128, 2048], bf16, kind="ExternalInput").ap()
        for t in "qkv"
    }
    w = {
        t: nc.dram_tensor(f"w{t}", [128, 512], bf16, kind="ExternalInput").ap()
        for t in "qkv"
    }
    bvec = {
        t: nc.dram_tensor(f"b{t}", [128, 1], f32, kind="ExternalInput").ap()
        for t in "qkv"
    }
    wo = nc.dram_tensor("wo", [128, 512], bf16, kind="ExternalInput").ap()
    y_out = [
        nc.dram_tensor(f"y{h}", [4, NQC, 128, 512], bf16, kind="ExternalOutput").ap()
        for h in range(2)
    ]
    den_out = nc.dram_tensor("den", [2, n], f32, kind="ExternalOutput").ap()

    with tile.TileContext(nc) as tc:
        with (
            tc.tile_pool(name="consts", bufs=1) as consts,
            tc.tile_pool(name="xtp", bufs=5) as xtp,
            tc.tile_pool(name="persist", bufs=1) as persist,
            tc.tile_pool(name="ep", bufs=8) as ep,
            tc.tile_pool(name="psA", bufs=2, space="PSUM") as psA,
            tc.tile_pool(name="psB", bufs=2, space="PSUM") as psB,
            tc.tile_pool(name="psC", bufs=2, space="PSUM") as psC,
        ):
            wsb, bsb = {}, {}
            for t in "qkv":
                wsb[t] = consts.tile([128, 512], bf16, name=f"w{t}sb", tag=f"w{t}sb")
            wosb = consts.tile([128, 512], bf16, name="wosb", tag="wosb")
            # k-path first on the sync queue so the first projection can start
            # as early as possible; v/q/wo ride the scalar HWDGE queue.
            nc.sync.dma_start(out=wsb["k"], in_=w["k"])
            nc.scalar.dma_start(out=wsb["v"], in_=w["v"])
            nc.scalar.dma_start(out=wsb["q"], in_=w["q"])
            nc.scalar.dma_start(out=wosb, in_=wo)
            if not zero_bias:
                for t in "qkv":
                    bsb[t] = consts.tile([128, 1], f32, name=f"b{t}sb", tag=f"b{t}sb")
                    nc.sync.dma_start(out=bsb[t], in_=bvec[t])
            ident = consts.tile([128, 128], bf16, name="ident")
            make_identity(nc, ident)
            if dve_split:
                mask_t = consts.tile([128, 256], f32, name="mmask", tag="mmask")
                nc.vector.memset(mask_t.bitcast(i32), 0x007FFFFF)

            qt_t = [
                persist.tile([128, 512], bf16, name=f"qt{i}", tag=f"qt{i}")
                for i in range(NQC)
            ]
            kt_t = [
                persist.tile([128, 512], bf16, name=f"kt{i}", tag=f"kt{i}")
                for i in range(NQC)
            ]
            vt_t = [
                persist.tile([128, 512], bf16, name=f"vt{i}", tag=f"vt{i}")
                for i in range(NQC)
            ]
            # augmented V chunks: 64 head dims + ones column (col 64)
            vch = [
                [
                    persist.tile(
                        [128, 65], bf16, name=f"vch{h}_{c}", tag=f"vch{h}_{c}"
                    )
                    for c in range(NKC)
                ]
                for h in range(2)
            ]
            ot = [
                persist.tile([128, 512], bf16, name=f"ot{qc}", tag=f"ot{qc}")
                for qc in range(NQC)
            ]
            den_sb = [
                persist.tile([1, n], f32, name=f"den{h}", tag=f"den{h}")
                for h in range(2)
            ]
            for h in range(2):
                for c in range(NKC):
                    nc.vector.memset(vch[h][c][:, 64:65], 1.0)

            def proj(t, nk, dst, dma_eng=None):
                ppsum = psC.tile([128, 512], f32, name=f"pp_{t}{nk}", tag="y")
                xtile = xtp.tile([128, 2048], bf16, name=f"x_{t}{nk}", tag="xt")
                (dma_eng or nc.sync).dma_start(out=xtile, in_=xt[t][nk])
                for dc in range(4):
                    nc.tensor.matmul(
                        ppsum,
                        wsb[t][:, dc * 128 : (dc + 1) * 128],
                        xtile[:, dc * 512 : (dc + 1) * 512],
                        start=(dc == 0),
                        stop=(dc == 3),
                    )
                if zero_bias:
                    nc.scalar.activation(
                        out=dst, in_=ppsum, func=mybir.ActivationFunctionType.Copy
                    )
                else:
                    nc.vector.tensor_scalar_add(out=dst, in0=ppsum, scalar1=bsb[t])

            def vproj_block(nk):
                # V projection tile nk plus its per-head transposed chunks,
                # woven into attention qc0 (DMAs on the scalar HWDGE queue)
                proj("v", nk, vt_t[nk], dma_eng=nc.scalar)
                for c in range(nk * 4, nk * 4 + 4):
                    pt = psC.tile([128, 512], bf16, name=f"pt{c}", tag="y")
                    nc.tensor.transpose(
                        pt[:, 0:128],
                        vt_t[c // 4][:, (c % 4) * 128 : (c % 4 + 1) * 128],
                        ident,
                    )
                    for h in range(2):
                        nc.vector.tensor_copy(
                            out=vch[h][c][:, 0:64], in_=pt[:, h * 64 : (h + 1) * 64]
                        )

            # ---- phase 1: K projections only; V and Q projections are woven
            # into the attention stream ----
            for nk in range(NQC):
                proj("k", nk, kt_t[nk],
                     dma_eng=nc.scalar if nk % 2 else nc.sync)
            proj("q", 0, qt_t[0], dma_eng=nc.scalar)

            # ---- phase 2: attention + woven out-projection ----
            def outproj(qc):
                qs = slice(qc * 512, (qc + 1) * 512)
                for mt in range(4):
                    for h in range(2):
                        hp = slice(h * 64, (h + 1) * 64)
                        y_ps = psC.tile(
                            [128, 512], f32, name=f"y_{h}_{qc}_{mt}", tag="y"
                        )
                        nc.tensor.matmul(
                            y_ps,
                            wosb[hp, mt * 128 : (mt + 1) * 128],
                            ot[qc][hp, :],
                            start=True,
                            stop=True,
                            skip_group_check=True,
                        )
                        y_sb = xtp.tile(
                            [128, 512], bf16, name=f"ysb_{h}_{qc}_{mt}", tag="ysb"
                        )
                        if h == 0:
                            nc.vector.tensor_copy(out=y_sb, in_=y_ps)
                        else:
                            nc.scalar.copy(out=y_sb, in_=y_ps)
                        nc.sync.dma_start(out=y_out[h][mt, qc], in_=y_sb)

            for qc in range(NQC):
                qs = slice(qc * 512, (qc + 1) * 512)
                if qc == 0:
                    weave = {bi: bi // 2 for bi in range(0, 16, 2)}  # vproj nk
                else:
                    weave = {}
                o_ps = {
                    h: psB.tile([128, 512], f32, name=f"o_{h}_{qc}", tag="oy")
                    for h in range(2)
                }

                def emit_o(blk):
                    k0, blen, e_tiles = blk
                    for h in range(2):
                        for j in range(blen):
                            kc = k0 + j
                            nc.tensor.matmul(
                                o_ps[h][0:65, :],
                                vch[h][kc],
                                e_tiles[h][:, j * 512 : (j + 1) * 512],
                                start=(kc == 0),
                                stop=(kc == NKC - 1),
                                skip_group_check=True,
                            )

                pend = []
                for bi, (k0, blen) in enumerate(blocks):
                    if bi in weave:
                        vproj_block(weave[bi])
                    if qc == 0 and bi == 9:
                        proj("q", 1, qt_t[1], dma_eng=nc.scalar)
                    e_tiles = {}
                    for h in range(2):
                        hp = slice(h * 64, (h + 1) * 64)
                        s_ps = psA.tile(
                            [128, blen * 512], f32, name=f"s_{h}_{qc}_{k0}", tag="s"
                        )
                        for j in range(blen):
                            kc = k0 + j
                            nc.tensor.matmul(
                                s_ps[:, j * 512 : (j + 1) * 512],
                                kt_t[kc // 4][hp, (kc % 4) * 128 : (kc % 4 + 1) * 128],
                                qt_t[qc][hp, :],
                                start=True,
                                stop=True,
                                skip_group_check=True,
                            )
                        e_sb = ep.tile(
                            [128, blen * 512], bf16, name=f"e_{h}_{qc}_{k0}", tag="e"
                        )
                        u = act_cols(blen)
                        nc.scalar.activation(
                            e_sb[:, 0:u], s_ps[:, 0:u], Exp, scale=0.125
                        )
                        if u < blen * 512:
                            nc.vector.tensor_scalar(
                                out=e_sb[:, u:].bitcast(i16),
                                in0=s_ps[:, u:],
                                scalar1=_A_DVE,
                                scalar2=_B_DVE,
                                op0=mybir.AluOpType.mult,
                                op1=mybir.AluOpType.add,
                            )
                            nc.vector._custom_dve(
                                corr_op,
                                out=e_sb[:, u:],
                                in0=e_sb[:, u:],
                                in1=mask_t[:, 0 : blen * 512 - u],
                                s0=_CORR_A,
                                s1=_CORR_Q2,
                                imm2=_CORR_C,
                            )
                        e_tiles[h] = e_sb
                    pend.append((k0, blen, e_tiles))
                    if len(pend) > 2:
                        emit_o(pend.pop(0))
                for blk in pend:
                    emit_o(blk)

                for h in range(2):
                    nc.vector.tensor_copy(
                        out=ot[qc][h * 64 : (h + 1) * 64, :], in_=o_ps[h][0:64, :]
                    )
                    nc.vector.tensor_copy(
                        out=den_sb[h][0:1, qs], in_=o_ps[h][64:65, :]
                    )
                    nc.sync.dma_start(
                        out=den_out[h : h + 1, qs], in_=den_sb[h][0:1, qs]
                    )
                if qc + 2 < NQC:
                    proj("q", qc + 2, qt_t[qc + 2], dma_eng=nc.scalar)
                outproj(qc)

    nc.finalize()
    return nc


def get_nc(n=_N, zero_bias=False, dve_split=True):
    key = (n, zero_bias, dve_split)
    if key not in _nc_cache:
        _nc_cache[key] = _build(n, zero_bias, dve_split)
    return _nc_cache[key]


def make_in_maps(Q, K, V, Wq, bq, Wk, bk, Wv, bv, Wo, bo, n=_N):
    """Per-core input dicts (host-side sharding / layout prep)."""
    bf = np.float16
    nqc = n // 512
    xts = {}
    for b in range(_B):
        d = {}
        for t, X in (("q", Q), ("k", K), ("v", V)):
            xt = X[b][:n].T.astype(bf)  # [512, n]
            d[f"xt{t}"] = np.ascontiguousarray(
                xt.reshape(4, 128, nqc, 512).transpose(2, 1, 0, 3).reshape(nqc, 128, 2048)
            )
        xts[b] = d
    in_maps = []
    for c in range(_NCORES):
        b, p = divmod(c, 4)
        off = p * 128
        m = dict(xts[b])
        for t, W, bias in (("q", Wq, bq), ("k", Wk, bk), ("v", Wv, bv)):
            m[f"w{t}"] = np.ascontiguousarray(
                W[:, off : off + 128]
                .reshape(4, 128, 128)
                .transpose(1, 0, 2)
                .reshape(128, 512)
                .astype(bf)
            )
            m[f"b{t}"] = np.ascontiguousarray(bias[off : off + 128].reshape(128, 1))
        m["wo"] = np.ascontiguousarray(Wo[off : off + 128].astype(bf))
        in_maps.append(m)
    return in_maps


def assemble(results, bo, n=_N):
    """Cross-core reduction: normalize by softmax denominators, sum heads,
    add output bias, restore [B, N, D] layout."""
    nqc = n // 512
    out = np.empty((_B, n, _D), np.float32)
    for b in range(_B):
        acc = np.zeros((_D, n), np.float32)
        for p in range(4):
            r = results[4 * b + p]
            for h in range(2):
                # y [4, nqc, 128, 512] -> [512, n]
                y = (
                    r[f"y{h}"]
                    .astype(np.float32)
                    .transpose(0, 2, 1, 3)
                    .reshape(_D, n)
                )
                acc += y / r["den"][h][None, :]
        out[b] = acc.T + bo
    return out


def kernel(Q, K, V, Wq, bq, Wk, bk, Wv, bv, Wo, bo):
    from concourse import bass_utils

    args = [np.asarray(a, np.float32) for a in (Q, K, V, Wq, bq, Wk, bk, Wv, bv, Wo, bo)]
    Q, K, V, Wq, bq, Wk, bk, Wv, bv, Wo, bo = args
    zb = not (np.any(bq) or np.any(bk) or np.any(bv))
    nc = get_nc(zero_bias=zb)
    in_maps = make_in_maps(Q, K, V, Wq, bq, Wk, bk, Wv, bv, Wo, bo)
    res = bass_utils.run_bass_kernel_spmd(
        nc, in_maps, core_ids=list(range(_NCORES))
    )
    return assemble(res.results, bo)
